# revision 29
# baseline (speedup 1.0000x reference)
"""Masked multi-head attention on 8 TRN2 NeuronCores.

Sharding: 8 cores = 2 batches x 4 head-groups (4 heads of 64 dims each).
Each core computes full causal attention for its (batch, 4-head) slice.

v7 design:
  - bf16 projections (fp8 DoubleRow tested: 6e-2 rel err, over the gate).
  - V projected directly into key-major layout (X-tile stationary, Wv
    streaming) -> no DMA transposes; V bias applied post-normalization
    (out = num/den + bv, exact identity).
  - PV matmul keeps 64 ones-columns in v_aug: PSUM rows 64:128 of the
    accumulator hold the softmax denominator (64 copies). Normalize via
    cross-partition copy to base 0 + reciprocal_approx_fast + multiply.
  - Causal mask applied post-exp as bf16 0/1 multiply on SBUF.
  - Scores for iteration i+1 are emitted before PV of iteration i;
    projection filler interleaves between them, so TensorE works under
    the ScalarE exp stream. Group ping-pong phase order spreads filler.
  - DMAs: HWDGE transfers occupy the issuing engine's queue for their
    duration, so the SP queue carries wq + X + outputs and the ACT queue
    carries wk/wv/smalls strictly before the first ACTIVATE. X is one
    merged tile loaded with one big DMA per 512-query chunk.
"""
import threading
from collections import deque
from contextlib import ExitStack

import ml_dtypes
import numpy as np

import concourse.bass as bass
import concourse.tile as tile
from concourse import bacc, mybir
from concourse.bass_utils import run_bass_kernel_spmd

F32 = mybir.dt.float32
MMDT = mybir.dt.bfloat16
NPDT = ml_dtypes.bfloat16

B, T, C = 2, 2048, 1024
H, DH = 16, 64
HPC = 4            # heads per core
RPC = HPC * DH     # 256 output channels per core
NCT = C // 128     # 8 contraction tiles
NQC = T // 512     # 4 query chunks
NKT = T // 128     # 16 key tiles

QK_STEPS, V_STEPS = 9, 17


class Filler:
    """Queue of projection-step generators pulled as filler."""

    def __init__(self):
        self.q = deque()
        self.pulled = 0

    def add(self, *gens):
        self.q.extend(gens)

    def pull(self, n):
        done = 0
        while done < n and self.q:
            try:
                next(self.q[0])
                done += 1
                self.pulled += 1
            except StopIteration:
                self.q.popleft()

    def pull_to(self, total):
        self.pull(total - self.pulled)

    def drain(self):
        while self.q:
            self.pull(1 << 20)


def _build():
    nc = bacc.Bacc("TRN2", target_bir_lowering=False, debug=False)
    xt = nc.dram_tensor("xt", [128, NCT, T], MMDT,
                        kind="ExternalInput").ap()
    wq = nc.dram_tensor("wq", [C, RPC], MMDT, kind="ExternalInput").ap()
    wk = nc.dram_tensor("wk", [C, RPC], MMDT, kind="ExternalInput").ap()
    wv = nc.dram_tensor("wv", [C, RPC], MMDT, kind="ExternalInput").ap()
    bq = nc.dram_tensor("bq", [128, 2], F32, kind="ExternalInput").ap()
    bk = nc.dram_tensor("bk", [128, 2], F32, kind="ExternalInput").ap()
    bvc = nc.dram_tensor("bvc", [64, HPC], F32, kind="ExternalInput").ap()
    mask01 = nc.dram_tensor("mask01", [128, 128], MMDT,
                            kind="ExternalInput").ap()
    ot = nc.dram_tensor("ot", [RPC, T], F32, kind="ExternalOutput").ap()

    with tile.TileContext(nc) as tc, ExitStack() as ctx:
        per = ctx.enter_context(tc.tile_pool(name="per", bufs=1))
        wrk = ctx.enter_context(tc.tile_pool(name="wrk", bufs=4))
        tl = ctx.enter_context(tc.tile_pool(name="tl", bufs=2))
        ps = ctx.enter_context(tc.tile_pool(name="ps", bufs=1, space="PSUM"))

        wq_s = per.tile([128, NCT, RPC], MMDT, tag="wq")
        wk_s = per.tile([128, NCT, RPC], MMDT, tag="wk")
        wv_s = per.tile([128, NCT, RPC], MMDT, tag="wv")
        xt_s = per.tile([128, NCT, T], MMDT, tag="xt")

        # SP queue: X only (chunk 0 split in two so the first proj matmuls
        # can start after half of it). ACT queue: all weights + smalls,
        # strictly before the first ACTIVATE is emitted.
        nc.sync.dma_start(xt_s[:, 0:4, 0:512], xt[:, 0:4, 0:512])
        nc.scalar.dma_start(wq_s[:], wq.rearrange("(c p) m -> p c m", p=128))
        nc.sync.dma_start(xt_s[:, 4:8, 0:512], xt[:, 4:8, 0:512])
        nc.scalar.dma_start(wk_s[:], wk.rearrange("(c p) m -> p c m", p=128))
        bq_s = per.tile([128, 2], F32, tag="bq")
        bk_s = per.tile([128, 2], F32, tag="bk")
        bvc_s = per.tile([64, HPC], F32, tag="bvc")
        mask_s = per.tile([128, 128], MMDT, tag="mask")
        nc.scalar.dma_start(bq_s[:], bq[:])
        nc.scalar.dma_start(bk_s[:], bk[:])
        nc.scalar.dma_start(wv_s[:], wv.rearrange("(c p) m -> p c m", p=128))
        nc.scalar.dma_start(bvc_s[:], bvc[:])
        nc.scalar.dma_start(mask_s[:], mask01[:])
        for chk in range(1, NQC):
            nc.sync.dma_start(
                xt_s[:, :, 512 * chk:512 * (chk + 1)],
                xt[:, :, 512 * chk:512 * (chk + 1)])

        # V in key-major layout: [key-part, ktile, head, 64 v | 64 ones]
        v_aug = per.tile([128, NKT, HPC, 2 * DH], MMDT, tag="vaug")
        nc.gpsimd.memset(v_aug[:, :, :, DH:2 * DH], 1.0)

        # PE warm-up: ~3.4us of dummy matmuls during the DMA wait flips
        # the HAM clock gate to 2.4 GHz before the real projections start.
        warm = per.tile([128, 512], MMDT, tag="warm")
        nc.gpsimd.memset(warm[:], 0.0)
        wp = ps.tile([128, 512], F32, tag="pq", bufs=2, name="warmup")
        for _ in range(8):
            nc.tensor.matmul(wp[:], warm[:, 0:128], warm[:],
                             start=True, stop=True)

        qt_s = per.tile([128, 2, T], MMDT, tag="qt")
        kt_s = per.tile([128, 2, T], MMDT, tag="kt")

        # ---- projection step generators (one yield per engine op) ----
        def qk_steps(w_s, b_s, o_s, gr, chk):
            pq = ps.tile([128, 512], F32, tag="pq", bufs=2,
                         name=f"pq_{o_s.tensor.name}_{gr}_{chk}")
            for ct in range(NCT):
                nc.tensor.matmul(
                    pq[:],
                    w_s[:, ct, 128 * gr:128 * (gr + 1)],
                    xt_s[:, ct, 512 * chk:512 * (chk + 1)],
                    start=(ct == 0), stop=(ct == NCT - 1),
                )
                yield
            nc.vector.tensor_scalar_add(
                o_s[:, gr, 512 * chk:512 * (chk + 1)], pq[:],
                b_s[:, gr:gr + 1])
            yield

        def v_steps(tp):
            # t-tile pair tp: t-tiles 2tp, 2tp+1 -> v_aug[:, 2tp:2tp+2]
            pv = ps.tile([128, 512], F32, tag="pq", bufs=2, name=f"pv_{tp}")
            for i in range(2):
                tt = 2 * tp + i
                for ct in range(NCT):
                    nc.tensor.matmul(
                        pv[:, 256 * i:256 * (i + 1)],
                        xt_s[:, ct, 128 * tt:128 * (tt + 1)],
                        wv_s[:, ct, :],
                        start=(ct == 0), stop=(ct == NCT - 1),
                    )
                    yield
            nc.vector.tensor_copy(
                v_aug[:, 2 * tp:2 * tp + 2, :, 0:DH],
                pv[:].rearrange("p (a h d) -> p a h d", a=2, h=HPC, d=DH))
            yield

        # ---- attention building blocks ----
        def emit_scores(gr, chk, kt, s2s, e2s):
            q0 = 512 * chk
            diag = kt >= 4 * chk
            w0 = 128 * (kt - 4 * chk) if diag else 0
            s2 = ps.tile([128, 2, 512], F32, tag="s2", bufs=2)
            e2 = wrk.tile([128, 2, 512], MMDT, tag="e2")
            s2s[kt], e2s[kt] = s2, e2
            ksl = slice(128 * kt, 128 * (kt + 1))
            qsl = slice(q0 + w0, q0 + 512)
            nc.tensor.matmul(
                s2[:, 0, w0:512],
                kt_s[0:64, gr, ksl], qt_s[0:64, gr, qsl],
                start=True, stop=True,
            )
            nc.tensor.matmul(
                s2[:, 1, w0:512],
                kt_s[64:128, gr, ksl], qt_s[64:128, gr, qsl],
                start=True, stop=True,
            )
            if w0 >= 128:
                nc.scalar.activation(
                    e2[:, 0, w0:512], s2[:, 0, w0:512],
                    mybir.ActivationFunctionType.Exp)
                nc.scalar.activation(
                    e2[:, 1, w0:512], s2[:, 1, w0:512],
                    mybir.ActivationFunctionType.Exp)
            else:
                nc.scalar.activation(
                    e2[:], s2[:], mybir.ActivationFunctionType.Exp)
            if diag:
                mb = (mask_s[:].rearrange("p (o m) -> p o m", o=1)
                      .broadcast_to([128, 2, 128]))
                nc.vector.tensor_mul(
                    e2[:, :, w0:w0 + 128], e2[:, :, w0:w0 + 128], mb)

        def attn_chunk(gr, chk, filler, steps_left, pv_prereq=None,
                       pre=None, next_gc=None, last=False):
            hA, hB = 2 * gr, 2 * gr + 1
            q0 = 512 * chk
            ntA = ps.tile([128, 512], F32, tag="ntA", bufs=1)
            ntB = ps.tile([128, 512], F32, tag="ntB", bufs=1)
            nkt = 4 * chk + 4
            # front-load: aim to drain this phase's filler by ~60% through
            # the kt loop so phase transitions don't carry a remainder
            horizon = max(1, int(nkt * 0.6))
            base_pulled = filler.pulled

            s2s, e2s = (pre if pre else ({}, {}))

            if 0 not in e2s:
                emit_scores(gr, chk, 0, s2s, e2s)
            for kt in range(nkt):
                diag = kt >= 4 * chk
                w0 = 128 * (kt - 4 * chk) if diag else 0
                if kt + 1 < nkt and kt + 1 not in e2s:
                    emit_scores(gr, chk, kt + 1, s2s, e2s)
                target = -(-steps_left * min(kt + 1, horizon) // horizon)
                filler.pull_to(base_pulled + target)
                if pv_prereq:
                    filler.pull_to(base_pulled + pv_prereq.get(kt, 0))
                e2 = e2s.pop(kt)
                s2s.pop(kt)
                nc.tensor.matmul(
                    ntA[:, w0:512],
                    v_aug[:, kt, hA, :], e2[:, 0, w0:512],
                    start=(kt == 0), stop=(kt == nkt - 1),
                    skip_group_check=True,
                )
                nc.tensor.matmul(
                    ntB[:, w0:512],
                    v_aug[:, kt, hB, :], e2[:, 1, w0:512],
                    start=(kt == 0), stop=(kt == nkt - 1),
                    skip_group_check=True,
                )
            # lookahead: emit the next phase's first two score tiles so
            # the exp stream keeps running through the phase transition
            # (this phase's filler -- which includes the next phase's
            # projection deps -- has fully drained by now).
            next_pre = ({}, {})
            if next_gc is not None:
                filler.drain()
                for j in range(2):
                    emit_scores(next_gc[0], next_gc[1], j,
                                next_pre[0], next_pre[1])

            def norm_steps():
                for hh, nt in ((hA, ntA), (hB, ntB)):
                    # custom DVE ops only run at partition base 0: bring
                    # the denominator rows down with a plain
                    # cross-partition copy first. In the final phase the
                    # copy goes to the (idle) ScalarE so it overlaps the
                    # DVE chain of the other head.
                    dd = tl.tile([64, 512], F32, tag="dd", bufs=2)
                    if last:
                        nc.scalar.copy(dd[:], nt[64:128, :])
                    else:
                        nc.vector.tensor_copy(dd[:], nt[64:128, :])
                    yield
                    rb = tl.tile([64, 512], F32, tag="rb", bufs=2)
                    nc.vector.reciprocal_approx_fast(out=rb[:], in_=dd[:])
                    yield
                    oo = tl.tile([64, 512], F32, tag="oo", bufs=2)
                    nc.vector.tensor_mul(oo[:], nt[0:64, :], rb[:])
                    yield
                    nc.vector.tensor_scalar_add(oo[:], oo[:],
                                                bvc_s[:, hh:hh + 1])
                    yield
                    nc.sync.dma_start(
                        ot[64 * hh:64 * hh + 64, q0:q0 + 512], oo[:])
                    yield
            return norm_steps(), next_pre

        # ---- schedule: ping-pong groups, deps one phase ahead ----
        def q_(g, c):
            return qk_steps(wq_s, bq_s, qt_s, g, c)

        def k_(g, c):
            return qk_steps(wk_s, bk_s, kt_s, g, c)

        # prologue: deps of A(0,0)'s scores; v0/v1 ride as priority filler
        for g in (q_(0, 0), k_(0, 0)):
            for _ in g:
                pass

        filler = Filler()
        phase_fill = [
            ((0, 0), [v_steps(0), v_steps(1), q_(1, 0), k_(1, 0)],
             2 * QK_STEPS + 2 * V_STEPS),
            ((1, 0), [k_(0, 1), v_steps(2), v_steps(3), q_(0, 1)],
             2 * QK_STEPS + 2 * V_STEPS),
            ((0, 1), [k_(1, 1), q_(1, 1)], 2 * QK_STEPS),
            ((1, 1), [k_(0, 2), v_steps(4), v_steps(5), q_(0, 2)],
             2 * QK_STEPS + 2 * V_STEPS),
            ((0, 2), [k_(1, 2), q_(1, 2)], 2 * QK_STEPS),
            ((1, 2), [k_(0, 3), v_steps(6), v_steps(7), q_(0, 3)],
             2 * QK_STEPS + 2 * V_STEPS),
            ((0, 3), [k_(1, 3), q_(1, 3)], 2 * QK_STEPS),
            ((1, 3), [], 0),
        ]
        NORM_STEPS = 10
        norm = None
        pre = None
        for idx, ((g, c), gens, nsteps) in enumerate(phase_fill):
            # previous phase's norm chain runs first in this phase's
            # filler; it must be fully emitted before PV(0) reuses the
            # nt accumulators (pv_prereq guarantees emission order).
            prereq = {0: NORM_STEPS} if norm is not None else {}
            if norm is not None:
                filler.add(norm)
                nsteps += NORM_STEPS
            filler.add(*gens)
            if idx == 0:
                # v0/v1 ride in A(0,0)'s filler: PV(kt) reads v-pair
                # kt//2, whose write is filler step V_STEPS*(kt//2+1)
                prereq = {kt: V_STEPS * (kt // 2 + 1) for kt in range(4)}
            next_gc = (phase_fill[idx + 1][0]
                       if idx + 1 < len(phase_fill) else None)
            norm, pre = attn_chunk(g, c, filler, nsteps, prereq or None,
                                   pre=pre, next_gc=next_gc,
                                   last=(idx + 1 == len(phase_fill)))
            filler.drain()
        for _ in norm:
            pass

    nc.compile()
    return nc


_LOCK = threading.Lock()
_NC = None


def _get_nc():
    global _NC
    with _LOCK:
        if _NC is None:
            _NC = _build()
    return _NC


def _mask01_tile():
    kp = np.arange(128)[:, None]
    j = np.arange(128)[None, :]
    return np.where(j >= kp, 1.0, 0.0).astype(NPDT)


def _shard_inputs(X, Wq, bq, Wk, bk, Wv, bv):
    X = np.asarray(X, dtype=np.float32)
    Wq = np.asarray(Wq, dtype=np.float32)
    Wk = np.asarray(Wk, dtype=np.float32)
    Wv = np.asarray(Wv, dtype=np.float32)
    bq = np.asarray(bq, dtype=np.float32)
    bk = np.asarray(bk, dtype=np.float32)
    bv = np.asarray(bv, dtype=np.float32)
    s = np.float32(1.0 / np.sqrt(DH))
    mask = _mask01_tile()
    in_maps = []
    for core in range(8):
        b, g = divmod(core, 4)
        sl = slice(RPC * g, RPC * (g + 1))
        in_maps.append({
            # [128, NCT, T]: xt[p, ct, t] = X[t, 128*ct + p]
            "xt": np.ascontiguousarray(
                X[b].T.reshape(NCT, 128, T).transpose(1, 0, 2)).astype(NPDT),
            "wq": np.ascontiguousarray((Wq[sl] * s).T).astype(NPDT),
            "wk": np.ascontiguousarray(Wk[sl].T).astype(NPDT),
            "wv": np.ascontiguousarray(Wv[sl].T).astype(NPDT),
            "bq": np.ascontiguousarray((bq[sl] * s).reshape(2, 128).T),
            "bk": np.ascontiguousarray(bk[sl].reshape(2, 128).T),
            "bvc": np.ascontiguousarray(bv[sl].reshape(HPC, 64).T),
            "mask01": mask,
        })
    return in_maps


def kernel(X, Wq, bq, Wk, bk, Wv, bv):
    nc = _get_nc()
    in_maps = _shard_inputs(X, Wq, bq, Wk, bk, Wv, bv)
    res = run_bass_kernel_spmd(nc, in_maps, core_ids=list(range(8)))
    out = np.empty((B, T, C), dtype=np.float32)
    for core in range(8):
        b, g = divmod(core, 4)
        out[b, :, RPC * g:RPC * (g + 1)] = res.results[core]["ot"].T
    return out
